# revision 19
# baseline (speedup 1.0000x reference)
"""Trainium2 Bass kernel for nn_Affinity: M = relu(Xh ⊕ Yh + b1) @ W2 + b2.

Math (reference):
    Xh = X @ (W1[:, :C] @ Wsr).T          # [N1, H]
    Yh = Y @ (W1[:, C:] @ Wtg).T          # [N2, H]
    M[a, b] = sum_h W2[h] * relu(Xh[a, h] + Yh[b, h] + b1[h]) + b2

Sharding: rows of X (N1) split across 8 cores; each core computes a
[128, 1024] tile of M. No cross-core communication.

Per-core kernel design:
  - Host pre-folds weights: AxT = (W1[:,:C] @ Wsr).T, AyT = (W1[:,C:] @ Wtg).T.
  - PE computes XhT [h, a] and YhT [h, b] (h on partitions).
  - Main loop over a: V = relu(YhT + XhT[:, a]) via DVE tensor_scalar
    (bf16, 4x mode) and ACT activation (bias trick), split ~3:1.
  - Contraction over h via PE: lhsT is a sliding one-hot window holding
    W2 in the column matching row a, so each matmul accumulates row a of
    the output into PSUM at partition a. MODE:
      "full": M=128 windows, standard 128x128 array mode (serial PE).
      "col4": M=32 windows, 128x32 column-tiled mode (4 concurrent
              streams); every matmul in the program is col-tiled so the
              array mode never switches.
  - PSUM [128, 512] x2 evacuated once at the end (+b2) and DMA'd out.
"""

import sys

if "/opt/trn_rl_repo" not in sys.path:
    sys.path.insert(0, "/opt/trn_rl_repo")

import numpy as np
import ml_dtypes

import concourse.bass as bass
import concourse.bacc as bacc
import concourse.tile as tile
from concourse import mybir
from concourse.bass_utils import run_bass_kernel_spmd

N1, N2, C, H = 1024, 1024, 128, 256
NCORES = 8
P = N1 // NCORES  # 128 rows of X per core

F32 = mybir.dt.float32
BF16 = mybir.dt.bfloat16
BF16_NP = ml_dtypes.bfloat16

MODE = "col4"  # "full" | "col4"
# Measured per-V-tile cost (ns) on HW; used for greedy load balancing of
# the 256 V-tiles across the three elementwise-capable engines.
V_COST = {"D": 397, "A": 1143, "G": 1100}
USE_GPSIMD = True

_CACHE = {}

# One-hot window width: window buffer has W2 at column WMAX-1, zeros
# elsewhere; slice [WMAX-1-m : WMAX-1-m+M] puts W2 at local column m.
def _wmax():
    return 255 if MODE == "full" else 63


def _build_program():
    nc = bacc.Bacc("TRN2", debug=False)
    wmax = _wmax()

    xt = nc.dram_tensor("xt", [C, P], F32, kind="ExternalInput")
    yt = nc.dram_tensor("yt", [C, N2], F32, kind="ExternalInput")
    axt = nc.dram_tensor("axt", [C, H], F32, kind="ExternalInput")
    ayt = nc.dram_tensor("ayt", [C, H], F32, kind="ExternalInput")
    b1t = nc.dram_tensor("b1t", [2, C, 1], F32, kind="ExternalInput")
    zw = nc.dram_tensor("zw", [2, C, wmax], BF16, kind="ExternalInput")
    b2v = nc.dram_tensor("b2v", [P, 1], F32, kind="ExternalInput")
    m_out = nc.dram_tensor("m_out", [P, N2], F32, kind="ExternalOutput")

    AL = mybir.AluOpType

    with tile.TileContext(nc) as tc:
        with (
            tc.tile_pool(name="const", bufs=1) as const,
            tc.tile_pool(name="v", bufs=24) as vpool,
            tc.tile_pool(name="outp", bufs=2) as outp,
        ):
            xt_sb = const.tile([C, P], F32)
            nc.sync.dma_start(xt_sb[:], xt[:])
            yt_sb = const.tile([C, N2], F32)
            nc.sync.dma_start(yt_sb[:], yt[:])
            axt_sb = const.tile([C, H], F32)
            nc.sync.dma_start(axt_sb[:], axt[:])
            ayt_sb = const.tile([C, H], F32)
            nc.sync.dma_start(ayt_sb[:], ayt[:])
            b2_sb = const.tile([P, 1], F32)
            nc.sync.dma_start(b2_sb[:], b2v[:])

            b1_sb, zw_sb = [], []
            for t in range(2):
                b = const.tile([C, 1], F32, tag=f"b1_{t}", name=f"b1_sb{t}")
                nc.sync.dma_start(b[:], b1t[t])
                b1_sb.append(b)
                z = const.tile([C, wmax], BF16, tag=f"zw_{t}", name=f"zw_sb{t}")
                nc.sync.dma_start(z[:], zw[t])
                zw_sb.append(z)

            def prep_matmul(ps_ap, lhsT_ap, rhs_ap):
                # In col4 mode every matmul must be 128x32 col-tiled so
                # the PE array mode never switches mid-kernel.
                if MODE == "col4":
                    mtot = lhsT_ap.shape[1]
                    for mo in range(0, mtot, 32):
                        jj = (ps_ap.base_partition() + mo) // 32
                        nc.tensor.matmul(
                            ps_ap[mo : mo + 32, :],
                            lhsT_ap[:, mo : mo + 32],
                            rhs_ap,
                            start=True, stop=True,
                            tile_position=(0, 32 * (jj % 4)),
                        )
                else:
                    nc.tensor.matmul(
                        ps_ap, lhsT_ap, rhs_ap, start=True, stop=True
                    )

            # Prep phase uses its own PSUM pool, released before the main
            # loop (which needs all 8 banks in col4 mode).
            with tc.tile_pool(name="pst", bufs=2, space="PSUM") as pst:
                # XhT [h, a] per h-tile, with b1 folded in (f32: ACT bias
                # and DVE tensor_scalar per-partition operand must be f32).
                xhb_f32 = []
                for t in range(2):
                    ps = pst.tile([C, P], F32, tag="prep", name=f"ps_xh{t}")
                    prep_matmul(ps[:], axt_sb[:, t * 128 : (t + 1) * 128], xt_sb[:])
                    xf = const.tile([C, P], F32, tag=f"xhb_f32_{t}", name=f"xhb{t}")
                    nc.scalar.activation(
                        xf[:], ps[:], mybir.ActivationFunctionType.Identity,
                        bias=b1_sb[t][:, 0:1],
                    )
                    xhb_f32.append(xf)

                # YhT [h, b] per h-tile, bf16 (b1 folded into Xh side).
                # PSUM evacuation on ACT (ScalarE is closest to PSUM).
                yh = []
                for t in range(2):
                    ysb = const.tile([C, N2], BF16, tag=f"yh_{t}", name=f"yh{t}")
                    for half in range(2):
                        ps = pst.tile(
                            [C, 512], F32, tag="prep", name=f"ps_yh{t}{half}"
                        )
                        prep_matmul(
                            ps[:],
                            ayt_sb[:, t * 128 : (t + 1) * 128],
                            yt_sb[:, half * 512 : (half + 1) * 512],
                        )
                        nc.scalar.copy(
                            ysb[:, half * 512 : (half + 1) * 512], ps[:]
                        )
                    yh.append(ysb)

            with tc.tile_pool(name="pso", bufs=1, space="PSUM") as pso:
                if MODE == "col4":
                    # One PSUM bank per (col-group, half): each accumulation
                    # region exclusively owns a bank, so per-region
                    # start=True bank-clears are safe.
                    ps_out = [
                        [
                            pso.tile(
                                [128, 512], F32,
                                tag=f"pso_{j}_{h}", name=f"ps_out_{j}_{h}",
                            )
                            for h in range(2)
                        ]
                        for j in range(4)
                    ]
                else:
                    ps_out = [
                        pso.tile([128, 512], F32, tag=f"pso_{h}", name=f"ps_out_{h}")
                        for h in range(2)
                    ]

                # a-iteration order: in col4 mode group a's so consecutive
                # matmuls rotate through the 4 column groups.
                if MODE == "col4":
                    a_order = [32 * j + g for g in range(32) for j in range(4)]
                else:
                    a_order = list(range(128))
                a_chunk = 4

                # Greedy least-loaded assignment of V-tiles to engines.
                load = {"D": 0.0, "A": 0.0, "G": 0.0 if USE_GPSIMD else 1e18}

                def v_engine():
                    e = min(load, key=lambda k: load[k] + V_COST[k])
                    load[e] += V_COST[e]
                    return e

                first_a, last_a = a_order[0], a_order[-1]
                for ci in range(0, 128, a_chunk):
                    chunk = a_order[ci : ci + a_chunk]
                    vs = {}
                    for t in range(2):
                        for a in chunk:
                            v = vpool.tile([C, N2], BF16, tag="v", name=f"v_{t}_{a}")
                            eng = v_engine()
                            if eng == "A":
                                nc.scalar.activation(
                                    v[:], yh[t][:],
                                    mybir.ActivationFunctionType.Relu,
                                    bias=xhb_f32[t][:, a : a + 1],
                                )
                            else:
                                veng = nc.vector if eng == "D" else nc.gpsimd
                                veng.tensor_scalar(
                                    v[:], yh[t][:],
                                    xhb_f32[t][:, a : a + 1], 0.0,
                                    AL.add, AL.max,
                                )
                            vs[(t, a)] = v
                    for t in range(2):
                        for half in range(2):
                            for a in chunk:
                                if MODE == "col4":
                                    j, m = a // 32, a % 32
                                    nc.tensor.matmul(
                                        ps_out[j][half][32 * j : 32 * j + 32, :],
                                        zw_sb[t][:, 31 - m : 63 - m],
                                        vs[(t, a)][:, half * 512 : (half + 1) * 512],
                                        start=(m == 0 and t == 0),
                                        stop=(m == 31 and t == 1),
                                        skip_group_check=True,
                                        tile_position=(0, 32 * j),
                                    )
                                else:
                                    nc.tensor.matmul(
                                        ps_out[half][:, :],
                                        zw_sb[t][:, 127 - a : 255 - a],
                                        vs[(t, a)][:, half * 512 : (half + 1) * 512],
                                        start=(a == first_a and t == 0),
                                        stop=(a == last_a and t == 1),
                                        skip_group_check=True,
                                    )

                for half in range(2):
                    o = outp.tile([128, 512], F32, tag="o", name=f"o_{half}")
                    if MODE == "col4":
                        for j in range(4):
                            sl = slice(32 * j, 32 * j + 32)
                            if j % 2 == 0:
                                nc.vector.tensor_scalar_add(
                                    o[sl, :], ps_out[j][half][sl, :], b2_sb[sl, 0:1]
                                )
                            else:
                                nc.scalar.activation(
                                    o[sl, :], ps_out[j][half][sl, :],
                                    mybir.ActivationFunctionType.Identity,
                                    bias=b2_sb[sl, 0:1],
                                )
                    else:
                        nc.vector.tensor_scalar_add(
                            o[:], ps_out[half][:], b2_sb[:, 0:1]
                        )
                    nc.sync.dma_start(m_out[:, half * 512 : (half + 1) * 512], o[:])

    nc.compile()
    return nc


def _get_program():
    if "nc" not in _CACHE:
        _CACHE["nc"] = _build_program()
    return _CACHE["nc"]


def kernel(X, Y, Wsr, Wtg, W1, b1, W2, b2, _trace=False, _trace_kwargs=None):
    X = np.asarray(X, np.float32)
    Y = np.asarray(Y, np.float32)
    Wsr = np.asarray(Wsr, np.float32)
    Wtg = np.asarray(Wtg, np.float32)
    W1 = np.asarray(W1, np.float32)
    b1 = np.asarray(b1, np.float32)
    W2 = np.asarray(W2, np.float32)
    b2 = np.asarray(b2, np.float32)

    # Host-side weight folding (tiny: O(C^2 H)).
    wmax = _wmax()
    AxT = np.ascontiguousarray((W1[:, :C] @ Wsr).T)  # [C, H]
    AyT = np.ascontiguousarray((W1[:, C:] @ Wtg).T)  # [C, H]
    b1t = np.ascontiguousarray(b1.reshape(2, C, 1))
    Zw = np.zeros((2, C, wmax), BF16_NP)
    Zw[0, :, wmax // 2] = W2[0, :C].astype(BF16_NP)
    Zw[1, :, wmax // 2] = W2[0, C:].astype(BF16_NP)
    b2v = np.full((P, 1), b2[0], np.float32)
    XT = np.ascontiguousarray(X.T)  # [C, N1]
    YT = np.ascontiguousarray(Y.T)  # [C, N2]

    in_maps = [
        {
            "xt": np.ascontiguousarray(XT[:, c * P : (c + 1) * P]),
            "yt": YT,
            "axt": AxT,
            "ayt": AyT,
            "b1t": b1t,
            "zw": Zw,
            "b2v": b2v,
        }
        for c in range(NCORES)
    ]

    nc = _get_program()
    res = run_bass_kernel_spmd(
        nc, in_maps, list(range(NCORES)), trace=_trace,
        **(_trace_kwargs or {}),
    )
    _CACHE["last_results"] = res
    M = np.concatenate([res.results[c]["m_out"] for c in range(NCORES)], axis=0)
    return M.astype(np.float32)


# revision 20
# speedup vs baseline: 8.4349x; 8.4349x over previous
"""Trainium2 Bass kernel for nn_Affinity: M = relu(Xh ⊕ Yh + b1) @ W2 + b2.

Math (reference):
    Xh = X @ (W1[:, :C] @ Wsr).T          # [N1, H]
    Yh = Y @ (W1[:, C:] @ Wtg).T          # [N2, H]
    M[a, b] = sum_h W2[h] * relu(Xh[a, h] + Yh[b, h] + b1[h]) + b2

Sharding: rows of X (N1) split across 8 cores; each core computes a
[128, 1024] tile of M. No cross-core communication.

Per-core kernel design:
  - Host pre-folds weights: AxT = (W1[:,:C] @ Wsr).T, AyT = (W1[:,C:] @ Wtg).T.
  - PE computes XhT [h, a] and YhT [h, b] (h on partitions).
  - Main loop over a: V = relu(YhT + XhT[:, a]) via DVE tensor_scalar
    (bf16, 4x mode) and ACT activation (bias trick), split ~3:1.
  - Contraction over h via PE: lhsT is a sliding one-hot window holding
    W2 in the column matching row a, so each matmul accumulates row a of
    the output into PSUM at partition a. MODE:
      "full": M=128 windows, standard 128x128 array mode (serial PE).
      "col4": M=32 windows, 128x32 column-tiled mode (4 concurrent
              streams); every matmul in the program is col-tiled so the
              array mode never switches.
  - PSUM [128, 512] x2 evacuated once at the end (+b2) and DMA'd out.
"""

import sys

if "/opt/trn_rl_repo" not in sys.path:
    sys.path.insert(0, "/opt/trn_rl_repo")

import numpy as np
import ml_dtypes

import concourse.bass as bass
import concourse.bacc as bacc
import concourse.tile as tile
from concourse import mybir
from concourse.bass_utils import run_bass_kernel_spmd

N1, N2, C, H = 1024, 1024, 128, 256
NCORES = 8
P = N1 // NCORES  # 128 rows of X per core

F32 = mybir.dt.float32
BF16 = mybir.dt.bfloat16
BF16_NP = ml_dtypes.bfloat16

MODE = "col4"  # "full" | "col4"
# Measured per-V-tile cost (ns) on HW; used for greedy load balancing of
# the 256 V-tiles across the three elementwise-capable engines.
V_COST = {"D": 397, "A": 1143, "G": 1100}
USE_GPSIMD = False

_CACHE = {}

# One-hot window width: window buffer has W2 at column WMAX-1, zeros
# elsewhere; slice [WMAX-1-m : WMAX-1-m+M] puts W2 at local column m.
def _wmax():
    return 255 if MODE == "full" else 63


def _build_program():
    nc = bacc.Bacc("TRN2", debug=False)
    wmax = _wmax()

    xt = nc.dram_tensor("xt", [C, P], F32, kind="ExternalInput")
    yt = nc.dram_tensor("yt", [C, N2], F32, kind="ExternalInput")
    axt = nc.dram_tensor("axt", [C, H], F32, kind="ExternalInput")
    ayt = nc.dram_tensor("ayt", [C, H], F32, kind="ExternalInput")
    b1t = nc.dram_tensor("b1t", [2, C, 1], F32, kind="ExternalInput")
    zw = nc.dram_tensor("zw", [2, C, wmax], BF16, kind="ExternalInput")
    b2v = nc.dram_tensor("b2v", [P, 1], F32, kind="ExternalInput")
    m_out = nc.dram_tensor("m_out", [P, N2], F32, kind="ExternalOutput")

    AL = mybir.AluOpType

    with tile.TileContext(nc) as tc:
        with (
            tc.tile_pool(name="const", bufs=1) as const,
            tc.tile_pool(name="v", bufs=24) as vpool,
            tc.tile_pool(name="outp", bufs=2) as outp,
        ):
            xt_sb = const.tile([C, P], F32)
            nc.sync.dma_start(xt_sb[:], xt[:])
            yt_sb = const.tile([C, N2], F32)
            nc.sync.dma_start(yt_sb[:], yt[:])
            axt_sb = const.tile([C, H], F32)
            nc.sync.dma_start(axt_sb[:], axt[:])
            ayt_sb = const.tile([C, H], F32)
            nc.sync.dma_start(ayt_sb[:], ayt[:])
            b2_sb = const.tile([P, 1], F32)
            nc.sync.dma_start(b2_sb[:], b2v[:])

            b1_sb, zw_sb = [], []
            for t in range(2):
                b = const.tile([C, 1], F32, tag=f"b1_{t}", name=f"b1_sb{t}")
                nc.sync.dma_start(b[:], b1t[t])
                b1_sb.append(b)
                z = const.tile([C, wmax], BF16, tag=f"zw_{t}", name=f"zw_sb{t}")
                nc.sync.dma_start(z[:], zw[t])
                zw_sb.append(z)

            def prep_matmul(ps_ap, lhsT_ap, rhs_ap):
                # In col4 mode every matmul must be 128x32 col-tiled so
                # the PE array mode never switches mid-kernel.
                if MODE == "col4":
                    mtot = lhsT_ap.shape[1]
                    for mo in range(0, mtot, 32):
                        jj = (ps_ap.base_partition() + mo) // 32
                        nc.tensor.matmul(
                            ps_ap[mo : mo + 32, :],
                            lhsT_ap[:, mo : mo + 32],
                            rhs_ap,
                            start=True, stop=True,
                            tile_position=(0, 32 * (jj % 4)),
                        )
                else:
                    nc.tensor.matmul(
                        ps_ap, lhsT_ap, rhs_ap, start=True, stop=True
                    )

            # Prep phase uses its own PSUM pool, released before the main
            # loop (which needs all 8 banks in col4 mode).
            with tc.tile_pool(name="pst", bufs=2, space="PSUM") as pst:
                # XhT [h, a] per h-tile, with b1 folded in (f32: ACT bias
                # and DVE tensor_scalar per-partition operand must be f32).
                xhb_f32 = []
                for t in range(2):
                    ps = pst.tile([C, P], F32, tag="prep", name=f"ps_xh{t}")
                    prep_matmul(ps[:], axt_sb[:, t * 128 : (t + 1) * 128], xt_sb[:])
                    xf = const.tile([C, P], F32, tag=f"xhb_f32_{t}", name=f"xhb{t}")
                    nc.scalar.activation(
                        xf[:], ps[:], mybir.ActivationFunctionType.Identity,
                        bias=b1_sb[t][:, 0:1],
                    )
                    xhb_f32.append(xf)

                # YhT [h, b] per h-tile, bf16 (b1 folded into Xh side).
                # PSUM evacuation on ACT (ScalarE is closest to PSUM).
                yh = []
                for t in range(2):
                    ysb = const.tile([C, N2], BF16, tag=f"yh_{t}", name=f"yh{t}")
                    for half in range(2):
                        ps = pst.tile(
                            [C, 512], F32, tag="prep", name=f"ps_yh{t}{half}"
                        )
                        prep_matmul(
                            ps[:],
                            ayt_sb[:, t * 128 : (t + 1) * 128],
                            yt_sb[:, half * 512 : (half + 1) * 512],
                        )
                        nc.scalar.copy(
                            ysb[:, half * 512 : (half + 1) * 512], ps[:]
                        )
                    yh.append(ysb)

            with tc.tile_pool(name="pso", bufs=1, space="PSUM") as pso:
                if MODE == "col4":
                    # One PSUM bank per (col-group, half): each accumulation
                    # region exclusively owns a bank, so per-region
                    # start=True bank-clears are safe.
                    ps_out = [
                        [
                            pso.tile(
                                [128, 512], F32,
                                tag=f"pso_{j}_{h}", name=f"ps_out_{j}_{h}",
                            )
                            for h in range(2)
                        ]
                        for j in range(4)
                    ]
                else:
                    ps_out = [
                        pso.tile([128, 512], F32, tag=f"pso_{h}", name=f"ps_out_{h}")
                        for h in range(2)
                    ]

                # a-iteration order: in col4 mode group a's so consecutive
                # matmuls rotate through the 4 column groups.
                if MODE == "col4":
                    a_order = [32 * j + g for g in range(32) for j in range(4)]
                else:
                    a_order = list(range(128))
                a_chunk = 4

                # Greedy least-loaded assignment of V-tiles to engines.
                load = {"D": 0.0, "A": 0.0, "G": 0.0 if USE_GPSIMD else 1e18}

                def v_engine():
                    e = min(load, key=lambda k: load[k] + V_COST[k])
                    load[e] += V_COST[e]
                    return e

                first_a, last_a = a_order[0], a_order[-1]
                for ci in range(0, 128, a_chunk):
                    chunk = a_order[ci : ci + a_chunk]
                    vs = {}
                    for t in range(2):
                        for a in chunk:
                            v = vpool.tile([C, N2], BF16, tag="v", name=f"v_{t}_{a}")
                            eng = v_engine()
                            if eng == "A":
                                nc.scalar.activation(
                                    v[:], yh[t][:],
                                    mybir.ActivationFunctionType.Relu,
                                    bias=xhb_f32[t][:, a : a + 1],
                                )
                            else:
                                veng = nc.vector if eng == "D" else nc.gpsimd
                                veng.tensor_scalar(
                                    v[:], yh[t][:],
                                    xhb_f32[t][:, a : a + 1], 0.0,
                                    AL.add, AL.max,
                                )
                            vs[(t, a)] = v
                    for t in range(2):
                        for half in range(2):
                            for a in chunk:
                                if MODE == "col4":
                                    j, m = a // 32, a % 32
                                    nc.tensor.matmul(
                                        ps_out[j][half][32 * j : 32 * j + 32, :],
                                        zw_sb[t][:, 31 - m : 63 - m],
                                        vs[(t, a)][:, half * 512 : (half + 1) * 512],
                                        start=(m == 0 and t == 0),
                                        stop=(m == 31 and t == 1),
                                        skip_group_check=True,
                                        tile_position=(0, 32 * j),
                                    )
                                else:
                                    nc.tensor.matmul(
                                        ps_out[half][:, :],
                                        zw_sb[t][:, 127 - a : 255 - a],
                                        vs[(t, a)][:, half * 512 : (half + 1) * 512],
                                        start=(a == first_a and t == 0),
                                        stop=(a == last_a and t == 1),
                                        skip_group_check=True,
                                    )

                for half in range(2):
                    o = outp.tile([128, 512], F32, tag="o", name=f"o_{half}")
                    if MODE == "col4":
                        for j in range(4):
                            sl = slice(32 * j, 32 * j + 32)
                            if j % 2 == 0:
                                nc.vector.tensor_scalar_add(
                                    o[sl, :], ps_out[j][half][sl, :], b2_sb[sl, 0:1]
                                )
                            else:
                                nc.scalar.activation(
                                    o[sl, :], ps_out[j][half][sl, :],
                                    mybir.ActivationFunctionType.Identity,
                                    bias=b2_sb[sl, 0:1],
                                )
                    else:
                        nc.vector.tensor_scalar_add(
                            o[:], ps_out[half][:], b2_sb[:, 0:1]
                        )
                    nc.sync.dma_start(m_out[:, half * 512 : (half + 1) * 512], o[:])

    nc.compile()
    return nc


def _get_program():
    if "nc" not in _CACHE:
        _CACHE["nc"] = _build_program()
    return _CACHE["nc"]


def kernel(X, Y, Wsr, Wtg, W1, b1, W2, b2, _trace=False, _trace_kwargs=None):
    X = np.asarray(X, np.float32)
    Y = np.asarray(Y, np.float32)
    Wsr = np.asarray(Wsr, np.float32)
    Wtg = np.asarray(Wtg, np.float32)
    W1 = np.asarray(W1, np.float32)
    b1 = np.asarray(b1, np.float32)
    W2 = np.asarray(W2, np.float32)
    b2 = np.asarray(b2, np.float32)

    # Host-side weight folding (tiny: O(C^2 H)).
    wmax = _wmax()
    AxT = np.ascontiguousarray((W1[:, :C] @ Wsr).T)  # [C, H]
    AyT = np.ascontiguousarray((W1[:, C:] @ Wtg).T)  # [C, H]
    b1t = np.ascontiguousarray(b1.reshape(2, C, 1))
    Zw = np.zeros((2, C, wmax), BF16_NP)
    Zw[0, :, wmax // 2] = W2[0, :C].astype(BF16_NP)
    Zw[1, :, wmax // 2] = W2[0, C:].astype(BF16_NP)
    b2v = np.full((P, 1), b2[0], np.float32)
    XT = np.ascontiguousarray(X.T)  # [C, N1]
    YT = np.ascontiguousarray(Y.T)  # [C, N2]

    in_maps = [
        {
            "xt": np.ascontiguousarray(XT[:, c * P : (c + 1) * P]),
            "yt": YT,
            "axt": AxT,
            "ayt": AyT,
            "b1t": b1t,
            "zw": Zw,
            "b2v": b2v,
        }
        for c in range(NCORES)
    ]

    nc = _get_program()
    res = run_bass_kernel_spmd(
        nc, in_maps, list(range(NCORES)), trace=_trace,
        **(_trace_kwargs or {}),
    )
    _CACHE["last_results"] = res
    M = np.concatenate([res.results[c]["m_out"] for c in range(NCORES)], axis=0)
    return M.astype(np.float32)


# revision 21
# speedup vs baseline: 9.9165x; 1.1756x over previous
"""Trainium2 Bass kernel for nn_Affinity: M = relu(Xh ⊕ Yh + b1) @ W2 + b2.

Math (reference):
    Xh = X @ (W1[:, :C] @ Wsr).T          # [N1, H]
    Yh = Y @ (W1[:, C:] @ Wtg).T          # [N2, H]
    M[a, b] = sum_h W2[h] * relu(Xh[a, h] + Yh[b, h] + b1[h]) + b2

Sharding: rows of X (N1) split across 8 cores; each core computes a
[128, 1024] tile of M. No cross-core communication.

Per-core kernel design:
  - Host pre-folds weights: AxT = (W1[:,:C] @ Wsr).T, AyT = (W1[:,C:] @ Wtg).T.
  - PE computes XhT [h, a] and YhT [h, b] (h on partitions).
  - Main loop over a: V = relu(YhT + XhT[:, a]) via DVE tensor_scalar
    (bf16, 4x mode) and ACT activation (bias trick), split ~3:1.
  - Contraction over h via PE: lhsT is a sliding one-hot window holding
    W2 in the column matching row a, so each matmul accumulates row a of
    the output into PSUM at partition a. MODE:
      "full": M=128 windows, standard 128x128 array mode (serial PE).
      "col4": M=32 windows, 128x32 column-tiled mode (4 concurrent
              streams); every matmul in the program is col-tiled so the
              array mode never switches.
  - PSUM [128, 512] x2 evacuated once at the end (+b2) and DMA'd out.
"""

import sys

if "/opt/trn_rl_repo" not in sys.path:
    sys.path.insert(0, "/opt/trn_rl_repo")

import numpy as np
import ml_dtypes

import concourse.bass as bass
import concourse.bacc as bacc
import concourse.tile as tile
from concourse import mybir
from concourse.bass_utils import run_bass_kernel_spmd

N1, N2, C, H = 1024, 1024, 128, 256
NCORES = 8
P = N1 // NCORES  # 128 rows of X per core

F32 = mybir.dt.float32
BF16 = mybir.dt.bfloat16
BF16_NP = ml_dtypes.bfloat16

MODE = "col4"  # "full" | "col4"
# Measured per-V-tile cost (ns) on HW; used for greedy load balancing of
# the 256 V-tiles across the three elementwise-capable engines.
V_COST = {"D": 397, "A": 1143, "G": 1100}
USE_GPSIMD = False

_CACHE = {}

# One-hot window width: window buffer has W2 at column WMAX-1, zeros
# elsewhere; slice [WMAX-1-m : WMAX-1-m+M] puts W2 at local column m.
def _wmax():
    return 255 if MODE == "full" else 63


def _build_program():
    nc = bacc.Bacc("TRN2", debug=False)
    wmax = _wmax()

    xt = nc.dram_tensor("xt", [C, P], F32, kind="ExternalInput")
    yt = nc.dram_tensor("yt", [C, N2], F32, kind="ExternalInput")
    axt = nc.dram_tensor("axt", [C, H], F32, kind="ExternalInput")
    ayt = nc.dram_tensor("ayt", [C, H], F32, kind="ExternalInput")
    b1t = nc.dram_tensor("b1t", [2, C, 1], F32, kind="ExternalInput")
    zw = nc.dram_tensor("zw", [2, C, wmax], BF16, kind="ExternalInput")
    b2v = nc.dram_tensor("b2v", [P, 1], F32, kind="ExternalInput")
    m_out = nc.dram_tensor("m_out", [P, N2], F32, kind="ExternalOutput")

    AL = mybir.AluOpType

    with tile.TileContext(nc) as tc:
        with (
            tc.tile_pool(name="const", bufs=1) as const,
            tc.tile_pool(name="v", bufs=16) as vpool,
            tc.tile_pool(name="outp", bufs=2) as outp,
        ):
            xt_sb = const.tile([C, P], F32)
            nc.sync.dma_start(xt_sb[:], xt[:])
            yt_sb = const.tile([C, N2], F32)
            nc.sync.dma_start(yt_sb[:], yt[:])
            axt_sb = const.tile([C, H], F32)
            nc.sync.dma_start(axt_sb[:], axt[:])
            ayt_sb = const.tile([C, H], F32)
            nc.sync.dma_start(ayt_sb[:], ayt[:])
            b2_sb = const.tile([P, 1], F32)
            nc.sync.dma_start(b2_sb[:], b2v[:])

            b1_sb, zw_sb = [], []
            for t in range(2):
                b = const.tile([C, 1], F32, tag=f"b1_{t}", name=f"b1_sb{t}")
                nc.sync.dma_start(b[:], b1t[t])
                b1_sb.append(b)
                z = const.tile([C, wmax], BF16, tag=f"zw_{t}", name=f"zw_sb{t}")
                nc.sync.dma_start(z[:], zw[t])
                zw_sb.append(z)

            def prep_matmul(ps_ap, lhsT_ap, rhs_ap):
                # In col4 mode every matmul must be 128x32 col-tiled so
                # the PE array mode never switches mid-kernel.
                if MODE == "col4":
                    mtot = lhsT_ap.shape[1]
                    for mo in range(0, mtot, 32):
                        jj = (ps_ap.base_partition() + mo) // 32
                        nc.tensor.matmul(
                            ps_ap[mo : mo + 32, :],
                            lhsT_ap[:, mo : mo + 32],
                            rhs_ap,
                            start=True, stop=True,
                            tile_position=(0, 32 * (jj % 4)),
                        )
                else:
                    nc.tensor.matmul(
                        ps_ap, lhsT_ap, rhs_ap, start=True, stop=True
                    )

            # Prep phase uses its own PSUM pool, released before the main
            # loop (which needs all 8 banks in col4 mode).
            with tc.tile_pool(name="pst", bufs=2, space="PSUM") as pst:
                # XhT [h, a] per h-tile, with b1 folded in (f32: ACT bias
                # and DVE tensor_scalar per-partition operand must be f32).
                xhb_f32 = []
                for t in range(2):
                    ps = pst.tile([C, P], F32, tag="prep", name=f"ps_xh{t}")
                    prep_matmul(ps[:], axt_sb[:, t * 128 : (t + 1) * 128], xt_sb[:])
                    xf = const.tile([C, P], F32, tag=f"xhb_f32_{t}", name=f"xhb{t}")
                    nc.scalar.activation(
                        xf[:], ps[:], mybir.ActivationFunctionType.Identity,
                        bias=b1_sb[t][:, 0:1],
                    )
                    xhb_f32.append(xf)

                # YhT [h, b] per h-tile, bf16 (b1 folded into Xh side).
                # PSUM evacuation on ACT (ScalarE is closest to PSUM).
                yh = []
                for t in range(2):
                    ysb = const.tile([C, N2], BF16, tag=f"yh_{t}", name=f"yh{t}")
                    for half in range(2):
                        ps = pst.tile(
                            [C, 512], F32, tag="prep", name=f"ps_yh{t}{half}"
                        )
                        prep_matmul(
                            ps[:],
                            ayt_sb[:, t * 128 : (t + 1) * 128],
                            yt_sb[:, half * 512 : (half + 1) * 512],
                        )
                        nc.scalar.copy(
                            ysb[:, half * 512 : (half + 1) * 512], ps[:]
                        )
                    yh.append(ysb)

            with tc.tile_pool(name="pso", bufs=1, space="PSUM") as pso:
                if MODE == "col4":
                    # One PSUM bank per (col-group, half): each accumulation
                    # region exclusively owns a bank, so per-region
                    # start=True bank-clears are safe.
                    ps_out = [
                        [
                            pso.tile(
                                [128, 512], F32,
                                tag=f"pso_{j}_{h}", name=f"ps_out_{j}_{h}",
                            )
                            for h in range(2)
                        ]
                        for j in range(4)
                    ]
                else:
                    ps_out = [
                        pso.tile([128, 512], F32, tag=f"pso_{h}", name=f"ps_out_{h}")
                        for h in range(2)
                    ]

                # a-iteration order: in col4 mode group a's so consecutive
                # matmuls rotate through the 4 column groups.
                if MODE == "col4":
                    a_order = [32 * j + g for g in range(32) for j in range(4)]
                else:
                    a_order = list(range(128))
                a_chunk = 4

                # Greedy least-loaded assignment of V-tiles to engines.
                load = {"D": 0.0, "A": 0.0, "G": 0.0 if USE_GPSIMD else 1e18}

                def v_engine():
                    e = min(load, key=lambda k: load[k] + V_COST[k])
                    load[e] += V_COST[e]
                    return e

                first_a, last_a = a_order[0], a_order[-1]
                for ci in range(0, 128, a_chunk):
                    chunk = a_order[ci : ci + a_chunk]
                    vs = {}
                    for t in range(2):
                        for a in chunk:
                            v = vpool.tile([C, N2], BF16, tag="v", name=f"v_{t}_{a}")
                            eng = v_engine()
                            if eng == "A":
                                nc.scalar.activation(
                                    v[:], yh[t][:],
                                    mybir.ActivationFunctionType.Relu,
                                    bias=xhb_f32[t][:, a : a + 1],
                                )
                            else:
                                veng = nc.vector if eng == "D" else nc.gpsimd
                                veng.tensor_scalar(
                                    v[:], yh[t][:],
                                    xhb_f32[t][:, a : a + 1], 0.0,
                                    AL.add, AL.max,
                                )
                            vs[(t, a)] = v
                    for t in range(2):
                        for half in range(2):
                            for a in chunk:
                                if MODE == "col4":
                                    j, m = a // 32, a % 32
                                    nc.tensor.matmul(
                                        ps_out[j][half][32 * j : 32 * j + 32, :],
                                        zw_sb[t][:, 31 - m : 63 - m],
                                        vs[(t, a)][:, half * 512 : (half + 1) * 512],
                                        start=(m == 0 and t == 0),
                                        stop=(m == 31 and t == 1),
                                        skip_group_check=True,
                                        tile_position=(0, 32 * j),
                                    )
                                else:
                                    nc.tensor.matmul(
                                        ps_out[half][:, :],
                                        zw_sb[t][:, 127 - a : 255 - a],
                                        vs[(t, a)][:, half * 512 : (half + 1) * 512],
                                        start=(a == first_a and t == 0),
                                        stop=(a == last_a and t == 1),
                                        skip_group_check=True,
                                    )

                for half in range(2):
                    o = outp.tile([128, 512], F32, tag="o", name=f"o_{half}")
                    if MODE == "col4":
                        for j in range(4):
                            sl = slice(32 * j, 32 * j + 32)
                            if j % 2 == 0:
                                nc.vector.tensor_scalar_add(
                                    o[sl, :], ps_out[j][half][sl, :], b2_sb[sl, 0:1]
                                )
                            else:
                                nc.scalar.activation(
                                    o[sl, :], ps_out[j][half][sl, :],
                                    mybir.ActivationFunctionType.Identity,
                                    bias=b2_sb[sl, 0:1],
                                )
                    else:
                        nc.vector.tensor_scalar_add(
                            o[:], ps_out[half][:], b2_sb[:, 0:1]
                        )
                    nc.sync.dma_start(m_out[:, half * 512 : (half + 1) * 512], o[:])

    nc.compile()
    return nc


def _get_program():
    if "nc" not in _CACHE:
        _CACHE["nc"] = _build_program()
    return _CACHE["nc"]


def kernel(X, Y, Wsr, Wtg, W1, b1, W2, b2, _trace=False, _trace_kwargs=None):
    X = np.asarray(X, np.float32)
    Y = np.asarray(Y, np.float32)
    Wsr = np.asarray(Wsr, np.float32)
    Wtg = np.asarray(Wtg, np.float32)
    W1 = np.asarray(W1, np.float32)
    b1 = np.asarray(b1, np.float32)
    W2 = np.asarray(W2, np.float32)
    b2 = np.asarray(b2, np.float32)

    # Host-side weight folding (tiny: O(C^2 H)).
    wmax = _wmax()
    AxT = np.ascontiguousarray((W1[:, :C] @ Wsr).T)  # [C, H]
    AyT = np.ascontiguousarray((W1[:, C:] @ Wtg).T)  # [C, H]
    b1t = np.ascontiguousarray(b1.reshape(2, C, 1))
    Zw = np.zeros((2, C, wmax), BF16_NP)
    Zw[0, :, wmax // 2] = W2[0, :C].astype(BF16_NP)
    Zw[1, :, wmax // 2] = W2[0, C:].astype(BF16_NP)
    b2v = np.full((P, 1), b2[0], np.float32)
    XT = np.ascontiguousarray(X.T)  # [C, N1]
    YT = np.ascontiguousarray(Y.T)  # [C, N2]

    in_maps = [
        {
            "xt": np.ascontiguousarray(XT[:, c * P : (c + 1) * P]),
            "yt": YT,
            "axt": AxT,
            "ayt": AyT,
            "b1t": b1t,
            "zw": Zw,
            "b2v": b2v,
        }
        for c in range(NCORES)
    ]

    nc = _get_program()
    res = run_bass_kernel_spmd(
        nc, in_maps, list(range(NCORES)), trace=_trace,
        **(_trace_kwargs or {}),
    )
    _CACHE["last_results"] = res
    M = np.concatenate([res.results[c]["m_out"] for c in range(NCORES)], axis=0)
    return M.astype(np.float32)


# revision 23
# speedup vs baseline: 10.0107x; 1.0095x over previous
"""Trainium2 Bass kernel for nn_Affinity: M = relu(Xh ⊕ Yh + b1) @ W2 + b2.

Math (reference):
    Xh = X @ (W1[:, :C] @ Wsr).T          # [N1, H]
    Yh = Y @ (W1[:, C:] @ Wtg).T          # [N2, H]
    M[a, b] = sum_h W2[h] * relu(Xh[a, h] + Yh[b, h] + b1[h]) + b2

Sharding: rows of X (N1) split across 8 cores; each core computes a
[128, 1024] tile of M. No cross-core communication.

Per-core kernel design:
  - Host pre-folds weights: AxT = (W1[:,:C] @ Wsr).T, AyT = (W1[:,C:] @ Wtg).T.
  - PE computes XhT [h, a] and YhT [h, b] (h on partitions).
  - Main loop over a: V = relu(YhT + XhT[:, a]) via DVE tensor_scalar
    (bf16, 4x mode) and ACT activation (bias trick), split ~3:1.
  - Contraction over h via PE: lhsT is a sliding one-hot window holding
    W2 in the column matching row a, so each matmul accumulates row a of
    the output into PSUM at partition a. MODE:
      "full": M=128 windows, standard 128x128 array mode (serial PE).
      "col4": M=32 windows, 128x32 column-tiled mode (4 concurrent
              streams); every matmul in the program is col-tiled so the
              array mode never switches.
  - PSUM [128, 512] x2 evacuated once at the end (+b2) and DMA'd out.
"""

import sys

if "/opt/trn_rl_repo" not in sys.path:
    sys.path.insert(0, "/opt/trn_rl_repo")

import numpy as np
import ml_dtypes

import concourse.bass as bass
import concourse.bacc as bacc
import concourse.tile as tile
from concourse import mybir
from concourse.bass_utils import run_bass_kernel_spmd

N1, N2, C, H = 1024, 1024, 128, 256
NCORES = 8
P = N1 // NCORES  # 128 rows of X per core

F32 = mybir.dt.float32
BF16 = mybir.dt.bfloat16
BF16_NP = ml_dtypes.bfloat16

MODE = "col4"  # "full" | "col4"
# Measured per-V-tile cost (ns) on HW; used for greedy load balancing of
# the 256 V-tiles across the three elementwise-capable engines.
V_COST = {"D": 397, "A": 1143, "G": 1100}
USE_GPSIMD = False

_CACHE = {}

# One-hot window width: window buffer has W2 at column WMAX-1, zeros
# elsewhere; slice [WMAX-1-m : WMAX-1-m+M] puts W2 at local column m.
def _wmax():
    return 255 if MODE == "full" else 63


def _build_program():
    nc = bacc.Bacc("TRN2", debug=False)
    wmax = _wmax()

    xt = nc.dram_tensor("xt", [C, P], F32, kind="ExternalInput")
    yt = nc.dram_tensor("yt", [C, N2], F32, kind="ExternalInput")
    axt = nc.dram_tensor("axt", [C, H], F32, kind="ExternalInput")
    ayt = nc.dram_tensor("ayt", [C, H], F32, kind="ExternalInput")
    b1t = nc.dram_tensor("b1t", [2, C, 1], F32, kind="ExternalInput")
    zw = nc.dram_tensor("zw", [2, C, wmax], BF16, kind="ExternalInput")
    b2v = nc.dram_tensor("b2v", [P, 1], F32, kind="ExternalInput")
    m_out = nc.dram_tensor("m_out", [P, N2], F32, kind="ExternalOutput")

    AL = mybir.AluOpType

    with tile.TileContext(nc) as tc:
        with (
            tc.tile_pool(name="const", bufs=1) as const,
            tc.tile_pool(name="v", bufs=16) as vpool,
            tc.tile_pool(name="outp", bufs=2) as outp,
        ):
            xt_sb = const.tile([C, P], F32)
            nc.sync.dma_start(xt_sb[:], xt[:])
            yt_sb = const.tile([C, N2], F32)
            nc.sync.dma_start(yt_sb[:], yt[:])
            axt_sb = const.tile([C, H], F32)
            nc.sync.dma_start(axt_sb[:], axt[:])
            ayt_sb = const.tile([C, H], F32)
            nc.sync.dma_start(ayt_sb[:], ayt[:])
            b2_sb = const.tile([P, 1], F32)
            nc.sync.dma_start(b2_sb[:], b2v[:])

            b1_sb, zw_sb = [], []
            for t in range(2):
                b = const.tile([C, 1], F32, tag=f"b1_{t}", name=f"b1_sb{t}")
                nc.sync.dma_start(b[:], b1t[t])
                b1_sb.append(b)
                z = const.tile([C, wmax], BF16, tag=f"zw_{t}", name=f"zw_sb{t}")
                nc.sync.dma_start(z[:], zw[t])
                zw_sb.append(z)

            def prep_matmul(ps_ap, lhsT_ap, rhs_ap):
                # In col4 mode every matmul must be 128x32 col-tiled so
                # the PE array mode never switches mid-kernel.
                if MODE == "col4":
                    mtot = lhsT_ap.shape[1]
                    for mo in range(0, mtot, 32):
                        jj = (ps_ap.base_partition() + mo) // 32
                        nc.tensor.matmul(
                            ps_ap[mo : mo + 32, :],
                            lhsT_ap[:, mo : mo + 32],
                            rhs_ap,
                            start=True, stop=True,
                            tile_position=(0, 32 * (jj % 4)),
                        )
                else:
                    nc.tensor.matmul(
                        ps_ap, lhsT_ap, rhs_ap, start=True, stop=True
                    )

            # Prep phase uses its own PSUM pool, released before the main
            # loop (which needs all 8 banks in col4 mode).
            with tc.tile_pool(name="pst", bufs=2, space="PSUM") as pst:
                # XhT [h, a] per h-tile, with b1 folded in (f32: ACT bias
                # and DVE tensor_scalar per-partition operand must be f32).
                xhb_f32 = []
                for t in range(2):
                    ps = pst.tile([C, P], F32, tag="prep", name=f"ps_xh{t}")
                    prep_matmul(ps[:], axt_sb[:, t * 128 : (t + 1) * 128], xt_sb[:])
                    xf = const.tile([C, P], F32, tag=f"xhb_f32_{t}", name=f"xhb{t}")
                    nc.scalar.activation(
                        xf[:], ps[:], mybir.ActivationFunctionType.Identity,
                        bias=b1_sb[t][:, 0:1],
                    )
                    xhb_f32.append(xf)

                # YhT [h, b] per h-tile, bf16 (b1 folded into Xh side).
                # PSUM evacuation on ACT (ScalarE is closest to PSUM).
                yh = []
                for t in range(2):
                    ysb = const.tile([C, N2], BF16, tag=f"yh_{t}", name=f"yh{t}")
                    for half in range(2):
                        ps = pst.tile(
                            [C, 512], F32, tag="prep", name=f"ps_yh{t}{half}"
                        )
                        prep_matmul(
                            ps[:],
                            ayt_sb[:, t * 128 : (t + 1) * 128],
                            yt_sb[:, half * 512 : (half + 1) * 512],
                        )
                        nc.scalar.copy(
                            ysb[:, half * 512 : (half + 1) * 512], ps[:]
                        )
                    yh.append(ysb)

            with tc.tile_pool(name="pso", bufs=1, space="PSUM") as pso:
                if MODE == "col4":
                    # One PSUM bank per (col-group, half): each accumulation
                    # region exclusively owns a bank, so per-region
                    # start=True bank-clears are safe.
                    ps_out = [
                        [
                            pso.tile(
                                [128, 512], F32,
                                tag=f"pso_{j}_{h}", name=f"ps_out_{j}_{h}",
                            )
                            for h in range(2)
                        ]
                        for j in range(4)
                    ]
                else:
                    ps_out = [
                        pso.tile([128, 512], F32, tag=f"pso_{h}", name=f"ps_out_{h}")
                        for h in range(2)
                    ]

                # a-iteration order: in col4 mode group a's so consecutive
                # matmuls rotate through the 4 column groups.
                if MODE == "col4":
                    a_order = [32 * j + g for g in range(32) for j in range(4)]
                else:
                    a_order = list(range(128))
                a_chunk = 4

                # Greedy least-loaded assignment of V-tiles to engines.
                load = {"D": 0.0, "A": 0.0, "G": 0.0 if USE_GPSIMD else 1e18}

                def v_engine():
                    e = min(load, key=lambda k: load[k] + V_COST[k])
                    load[e] += V_COST[e]
                    return e

                first_a, last_a = a_order[0], a_order[-1]
                for ci in range(0, 128, a_chunk):
                    chunk = a_order[ci : ci + a_chunk]
                    vs = {}
                    for t in range(2):
                        for a in chunk:
                            v = vpool.tile([C, N2], BF16, tag="v", name=f"v_{t}_{a}")
                            eng = v_engine()
                            if eng == "A":
                                nc.scalar.activation(
                                    v[:], yh[t][:],
                                    mybir.ActivationFunctionType.Relu,
                                    bias=xhb_f32[t][:, a : a + 1],
                                )
                            else:
                                veng = nc.vector if eng == "D" else nc.gpsimd
                                veng.tensor_scalar(
                                    v[:], yh[t][:],
                                    xhb_f32[t][:, a : a + 1], 0.0,
                                    AL.add, AL.max,
                                )
                            vs[(t, a)] = v
                    for t in range(2):
                        for half in range(2):
                            for a in chunk:
                                if MODE == "col4":
                                    j, m = a // 32, a % 32
                                    nc.tensor.matmul(
                                        ps_out[j][half][32 * j : 32 * j + 32, :],
                                        zw_sb[t][:, 31 - m : 63 - m],
                                        vs[(t, a)][:, half * 512 : (half + 1) * 512],
                                        start=(m == 0 and t == 0),
                                        stop=(m == 31 and t == 1),
                                        skip_group_check=True,
                                        tile_position=(0, 32 * j),
                                    )
                                else:
                                    nc.tensor.matmul(
                                        ps_out[half][:, :],
                                        zw_sb[t][:, 127 - a : 255 - a],
                                        vs[(t, a)][:, half * 512 : (half + 1) * 512],
                                        start=(a == first_a and t == 0),
                                        stop=(a == last_a and t == 1),
                                        skip_group_check=True,
                                    )

                for half in range(2):
                    o = outp.tile([128, 512], F32, tag="o", name=f"o_{half}")
                    if MODE == "col4":
                        for j in range(4):
                            sl = slice(32 * j, 32 * j + 32)
                            if j % 2 == 0:
                                nc.vector.tensor_scalar_add(
                                    o[sl, :], ps_out[j][half][sl, :], b2_sb[sl, 0:1]
                                )
                            else:
                                nc.scalar.activation(
                                    o[sl, :], ps_out[j][half][sl, :],
                                    mybir.ActivationFunctionType.Identity,
                                    bias=b2_sb[sl, 0:1],
                                )
                    else:
                        nc.vector.tensor_scalar_add(
                            o[:], ps_out[half][:], b2_sb[:, 0:1]
                        )
                    nc.sync.dma_start(m_out[:, half * 512 : (half + 1) * 512], o[:])

    nc.compile()
    return nc


def _get_program():
    if "nc" not in _CACHE:
        _CACHE["nc"] = _build_program()
    return _CACHE["nc"]


def kernel(X, Y, Wsr, Wtg, W1, b1, W2, b2, _trace=False, _trace_kwargs=None):
    X = np.asarray(X, np.float32)
    Y = np.asarray(Y, np.float32)
    Wsr = np.asarray(Wsr, np.float32)
    Wtg = np.asarray(Wtg, np.float32)
    W1 = np.asarray(W1, np.float32)
    b1 = np.asarray(b1, np.float32)
    W2 = np.asarray(W2, np.float32)
    b2 = np.asarray(b2, np.float32)

    # Host-side weight folding (tiny: O(C^2 H)).
    wmax = _wmax()
    AxT = np.ascontiguousarray((W1[:, :C] @ Wsr).T)  # [C, H]
    AyT = np.ascontiguousarray((W1[:, C:] @ Wtg).T)  # [C, H]
    b1t = np.ascontiguousarray(b1.reshape(2, C, 1))
    Zw = np.zeros((2, C, wmax), BF16_NP)
    Zw[0, :, wmax // 2] = W2[0, :C].astype(BF16_NP)
    Zw[1, :, wmax // 2] = W2[0, C:].astype(BF16_NP)
    b2v = np.full((P, 1), b2[0], np.float32)
    XT = np.ascontiguousarray(X.T)  # [C, N1]
    YT = np.ascontiguousarray(Y.T)  # [C, N2]

    in_maps = [
        {
            "xt": np.ascontiguousarray(XT[:, c * P : (c + 1) * P]),
            "yt": YT,
            "axt": AxT,
            "ayt": AyT,
            "b1t": b1t,
            "zw": Zw,
            "b2v": b2v,
        }
        for c in range(NCORES)
    ]

    nc = _get_program()
    res = run_bass_kernel_spmd(
        nc, in_maps, list(range(NCORES)), trace=_trace,
        **(_trace_kwargs or {}),
    )
    _CACHE["last_results"] = res
    M = np.concatenate([res.results[c]["m_out"] for c in range(NCORES)], axis=0)
    return M.astype(np.float32)
